# revision 1
# baseline (speedup 1.0000x reference)
"""DeepSeek-V3.2 MLA attention on 8 Trainium2 NeuronCores (Bass/Tile).

Strategy (tensor parallel over heads, per the sharding hint):
  Launch A: sequence-sharded latent projections. Core c computes the
    q/kv down-projections + RMSNorm for its 256-token slice, in
    feature-major ("transposed") layout so no on-chip transposes are
    needed anywhere. Host gathers the 8 slices.
  Launch B: head-sharded attention. Core c owns heads (2c, 2c+1): up-
    projections, Q@K^T (computed transposed: [k, q]), mask add, exp,
    denominator via ones-matmul on the PE, P@V, o-proj partial. Host
    sums the 8 partial outputs (the all-reduce after o_proj).

All matmul operands are float32r (TF32-like FP22 multiply, fp32
accumulate) which runs the PE at full bf16 rate for moving dims >= 256.

Host-side precomputation folds gqa/gkva into Wqb/Wkvb rows, the softmax
1/sqrt(192) into the q-latent normalization, and transposes
hidden_states / attention_mask (layout prep only - all FLOPs of the
module run on device).
"""

import numpy as np

import concourse.bass as bass
import concourse.tile as tile
from concourse import bacc, mybir
from concourse.bass_utils import run_bass_kernel_spmd

F32 = mybir.dt.float32
F32R = mybir.dt.float32r
BF16 = mybir.dt.bfloat16

S = 2048
HID = 2048
QL = 1536
KVL = 512
ROPE = 64
NOPE = 128
VH = 128
NH = 16
NCORES = 8
HPC = NH // NCORES          # heads per core = 2
SL = S // NCORES            # token slice per core in launch A = 256
QLT = QL // 128             # 12
KVT = KVL // 128            # 4
HT = HID // 128             # 16
ST = S // 128               # 16
EPS = 1e-6

_CACHE = {}


def _ap_slices(t, n, w):
    """Free-dim slices [i*w:(i+1)*w) of a [128, n*w] tile."""
    return [t[:, i * w:(i + 1) * w] for i in range(n)]


def _build_a():
    """Launch A: latents for a 256-token slice, feature-major.

    in : hsl [HID, SL] (hidden^T slice), wqa [HID, QL], wkva [HID, KVL+ROPE]
    out: q_lat [QL, SL]  = rmsnorm(hidden@Wqa) / sqrt(192)   (g folded later)
         kv_lat [KVL, SL] = rmsnorm-normalized kv latent
         rp_lat [ROPE, SL] = raw shared k_rope
    """
    nc = bacc.Bacc("TRN2", target_bir_lowering=False, debug=False,
                   num_devices=NCORES)
    hsl = nc.dram_tensor("hsl", [HID, SL], F32R, kind="ExternalInput").ap()
    wqa = nc.dram_tensor("wqa", [HID, QL], F32R, kind="ExternalInput").ap()
    wkva = nc.dram_tensor("wkva", [HID, KVL + ROPE], F32R,
                          kind="ExternalInput").ap()
    q_lat = nc.dram_tensor("q_lat", [QL, SL], F32, kind="ExternalOutput").ap()
    kv_lat = nc.dram_tensor("kv_lat", [KVL, SL], F32,
                            kind="ExternalOutput").ap()
    rp_lat = nc.dram_tensor("rp_lat", [ROPE, SL], F32,
                            kind="ExternalOutput").ap()

    with tile.TileContext(nc) as tc:
        with tc.tile_pool(name="w", bufs=1) as wp, \
             tc.tile_pool(name="h", bufs=1) as hp, \
             tc.tile_pool(name="lat", bufs=1) as lp, \
             tc.tile_pool(name="tmp", bufs=3) as tp, \
             tc.tile_pool(name="ps", bufs=2, space="PSUM") as pp, \
             tc.tile_pool(name="pss", bufs=2, space="PSUM") as psp:
            ht = hp.tile([128, HT * SL], F32R, tag="ht")
            for j in range(HT):
                nc.sync.dma_start(ht[:, j * SL:(j + 1) * SL],
                                  hsl[j * 128:(j + 1) * 128, :])
            htt = _ap_slices(ht, HT, SL)
            # column-block-major weight layout: block m holds all 16 h-tiles
            # of Wqa[:, m*128:(m+1)*128] side by side -> one DMA per block,
            # so the m-loop matmuls start as soon as block m lands.
            wqa_s = wp.tile([128, HT * QL], F32R, tag="wqa")
            for m in range(QLT):
                dst = wqa_s[:, m * HT * 128:(m + 1) * HT * 128]
                nc.sync.dma_start(
                    dst.rearrange("p (j c) -> p j c", c=128),
                    wqa[:, m * 128:(m + 1) * 128]
                    .rearrange("(j p) c -> p j c", p=128))
            wkva_s = wp.tile([128, HT * (KVL + ROPE)], F32R, tag="wkva")
            for m in range(KVT):
                dst = wkva_s[:, m * HT * 128:(m + 1) * HT * 128]
                nc.sync.dma_start(
                    dst.rearrange("p (j c) -> p j c", c=128),
                    wkva[:, m * 128:(m + 1) * 128]
                    .rearrange("(j p) c -> p j c", p=128))
            wkvr = wp.tile([128, HT * ROPE], F32R, tag="wkvr")
            nc.sync.dma_start(
                wkvr[:].rearrange("p (j c) -> p j c", c=ROPE),
                wkva[:, KVL:].rearrange("(j p) c -> p j c", p=128))

            ones_f = wp.tile([128, 1], F32, tag="ones")
            nc.vector.memset(ones_f[:], 1.0)
            ones = ones_f[:].bitcast(F32R)
            epsq = wp.tile([1, 1], F32, tag="epsq")
            nc.vector.memset(epsq[:], 192.0 * EPS)
            epsk = wp.tile([1, 1], F32, tag="epsk")
            nc.vector.memset(epsk[:], EPS)

            def down_path(n_tiles, col_of, ssq_scale, eps_ap, out_dram, pfx):
                """Shared q/kv path: down-proj, ssq, rsqrt, normalize, store."""
                raw = []
                ps_ssq = psp.tile([1, SL], F32, tag="ssq")
                for m in range(n_tiles):
                    ps = pp.tile([128, SL], F32, tag="dps")
                    for j in range(HT):
                        nc.tensor.matmul(ps[:], col_of(j, m), htt[j],
                                         start=(j == 0), stop=(j == HT - 1))
                    r = lp.tile([128, SL], F32R, tag=f"raw{pfx}{m}")
                    nc.vector.tensor_copy(r[:], ps[:])
                    raw.append(r)
                    sq = tp.tile([128, SL], F32R, tag="sq")
                    nc.scalar.square(sq[:], ps[:])
                    nc.tensor.matmul(ps_ssq[:], ones, sq[:],
                                     start=(m == 0), stop=(m == n_tiles - 1))
                sd = tp.tile([1, SL], F32, tag="sd")
                nc.scalar.activation(sd[:], ps_ssq[:],
                                     mybir.ActivationFunctionType.Sqrt,
                                     bias=eps_ap[:], scale=ssq_scale)
                rr = tp.tile([1, SL], F32, tag="rr")
                nc.vector.reciprocal_approx_fast(rr[:], sd[:])
                rb = tp.tile([128, SL], F32, tag="rb")
                nc.gpsimd.partition_broadcast(rb[:], rr[:1])
                for m in range(n_tiles):
                    no = tp.tile([128, SL], F32, tag="no")
                    nc.vector.tensor_mul(no[:], raw[m][:], rb[:])
                    nc.sync.dma_start(out_dram[m * 128:(m + 1) * 128, :],
                                      no[:])

            # q: fold softmax scale 1/sqrt(192) into the rmsnorm scale:
            #   r = 1/sqrt(192*(ssq/QL + eps)) = 1/sqrt(ssq*(192/QL) + 192*eps)
            down_path(QLT, lambda j, m: wqa_s[:, (m * HT + j) * 128:
                                              (m * HT + j + 1) * 128],
                      192.0 / QL, epsq, q_lat, "q")
            down_path(KVT, lambda j, m: wkva_s[:, (m * HT + j) * 128:
                                               (m * HT + j + 1) * 128],
                      1.0 / KVL, epsk, kv_lat, "k")
            # raw shared rope part (no norm)
            ps = pp.tile([64, SL], F32, tag="rps")
            for j in range(HT):
                nc.tensor.matmul(
                    ps[:], wkvr[:, j * ROPE:(j + 1) * ROPE],
                    htt[j], start=(j == 0), stop=(j == HT - 1))
            ro = tp.tile([64, SL], F32, tag="ro")
            nc.vector.tensor_copy(ro[:], ps[:])
            nc.sync.dma_start(rp_lat[:, :], ro[:])
    nc.compile()
    return nc


def _build_b():
    """Launch B: 2 heads of attention + o-proj partial over the full seq.

    in : qlat [QL, S], kvlat [KVL, S], rp [ROPE, S]  (feature-major latents)
         maskT [S, S] (mask transposed: maskT[k, q]),
         wqn [QL, 2*128], wqr [QL, 2*64], wkn [KVL, 2*128], wkv [KVL, 2*128],
         wo [2*128, HID]
    out: part [S, HID] (this core's 2-head contribution to the output)
    """
    nc = bacc.Bacc("TRN2", target_bir_lowering=False, debug=False,
                   num_devices=NCORES)
    qlat = nc.dram_tensor("qlat", [QL, S], BF16,
                          kind="ExternalInput").ap()
    kvlat = nc.dram_tensor("kvlat", [KVL, S], BF16,
                           kind="ExternalInput").ap()
    rp = nc.dram_tensor("rp", [ROPE, S], BF16, kind="ExternalInput").ap()
    maskT = nc.dram_tensor("maskT", [S, S], BF16,
                           kind="ExternalInput").ap()
    wqn = nc.dram_tensor("wqn", [QL, HPC * NOPE], BF16,
                         kind="ExternalInput").ap()
    wqr = nc.dram_tensor("wqr", [QL, HPC * 64], BF16,
                         kind="ExternalInput").ap()
    wkn = nc.dram_tensor("wkn", [KVL, HPC * NOPE], BF16,
                         kind="ExternalInput").ap()
    wkv = nc.dram_tensor("wkv", [KVL, HPC * VH], BF16,
                         kind="ExternalInput").ap()
    wo = nc.dram_tensor("wo", [HPC * VH, HID], F32R,
                        kind="ExternalInput").ap()
    part = nc.dram_tensor("part", [S, HID], F32, kind="ExternalOutput").ap()

    CH = 512            # up-projection chunk (moving dim)
    NCH = S // CH       # 8
    QC = 512            # attention query chunk
    NQC = S // QC       # 4

    with tile.TileContext(nc) as tc:
        with tc.tile_pool(name="w", bufs=1) as wp, \
             tc.tile_pool(name="act", bufs=1) as ap_, \
             tc.tile_pool(name="lq", bufs=2) as lqp, \
             tc.tile_pool(name="msk", bufs=24) as mp, \
             tc.tile_pool(name="tmp", bufs=2) as tp, \
             tc.tile_pool(name="et", bufs=8) as ep, \
             tc.tile_pool(name="out", bufs=3) as op, \
             tc.tile_pool(name="ps", bufs=2, space="PSUM") as pp, \
             tc.tile_pool(name="psden", bufs=2, space="PSUM") as pdp, \
             tc.tile_pool(name="pso", bufs=2, space="PSUM") as pop:
            ones_f = wp.tile([128, 1], F32, tag="ones")
            nc.vector.memset(ones_f[:], 1.0)
            ones = ones_f[:].bitcast(F32R)
            zb = wp.tile([128, 1], F32, tag="zb")
            nc.vector.memset(zb[:], 0.0)

            # ---- persistent per-head activations (feature-major) ----
            qn_T = [ap_.tile([128, S], BF16, tag=f"qnT{h}", name=f"qnT{h}")
                    for h in range(HPC)]
            qr2_T = ap_.tile([128, S], BF16, tag="qr2T")
            kn_T = [ap_.tile([128, S], BF16, tag=f"knT{h}", name=f"knT{h}")
                    for h in range(HPC)]
            v2 = ap_.tile([128, ST * HPC * VH], BF16, tag="v2")
            kr2_T = ap_.tile([128, S], BF16, tag="kr2T")

            # ---- phase 1: up-projections, chunked over tokens ----
            def load_chunk(c):
                csl = slice(c * CH, (c + 1) * CH)
                lq = lqp.tile([128, QLT * CH], BF16, tag="lq", name="lq")
                for m in range(QLT):
                    nc.sync.dma_start(lq[:, m * CH:(m + 1) * CH],
                                      qlat[m * 128:(m + 1) * 128, csl])
                lk = lqp.tile([128, KVT * CH], BF16, tag="lk", name="lk")
                for m in range(KVT):
                    nc.sync.dma_start(lk[:, m * CH:(m + 1) * CH],
                                      kvlat[m * 128:(m + 1) * 128, csl])
                nc.sync.dma_start(kr2_T[0:64, csl], rp[:, csl])
                nc.sync.dma_start(kr2_T[64:128, csl], rp[:, csl])
                return lq, lk

            pend = load_chunk(0)
            # ---- weights to SBUF (emitted after first latent chunk so the
            # first up-projection matmuls start as early as possible) ----
            wqn_s = wp.tile([128, QLT * HPC * NOPE], BF16, tag="wqn")
            for m in range(QLT):
                nc.sync.dma_start(
                    wqn_s[:, m * HPC * NOPE:(m + 1) * HPC * NOPE],
                    wqn[m * 128:(m + 1) * 128, :])
            wqr_s = wp.tile([128, QLT * HPC * 64], BF16, tag="wqr")
            for m in range(QLT):
                nc.sync.dma_start(wqr_s[:, m * HPC * 64:(m + 1) * HPC * 64],
                                  wqr[m * 128:(m + 1) * 128, :])
            wkn_s = wp.tile([128, KVT * HPC * NOPE], BF16, tag="wkn")
            for m in range(KVT):
                nc.sync.dma_start(
                    wkn_s[:, m * HPC * NOPE:(m + 1) * HPC * NOPE],
                    wkn[m * 128:(m + 1) * 128, :])
            wkv_s = wp.tile([128, KVT * HPC * VH], BF16, tag="wkv")
            for m in range(KVT):
                nc.sync.dma_start(wkv_s[:, m * HPC * VH:(m + 1) * HPC * VH],
                                  wkv[m * 128:(m + 1) * 128, :])
            wo_s = wp.tile([128, HPC * HID], F32R, tag="wo")
            for h in range(HPC):
                nc.sync.dma_start(wo_s[:, h * HID:(h + 1) * HID],
                                  wo[h * 128:(h + 1) * 128, :])
            for c in range(NCH):
                csl = slice(c * CH, (c + 1) * CH)
                lq, lk = pend
                if c + 1 < NCH:
                    pend = load_chunk(c + 1)

                for h in range(HPC):
                    ps = pp.tile([128, CH], F32, tag="ups")
                    for m in range(QLT):
                        nc.tensor.matmul(
                            ps[:],
                            wqn_s[:, m * HPC * NOPE + h * NOPE:
                                  m * HPC * NOPE + (h + 1) * NOPE],
                            lq[:, m * CH:(m + 1) * CH],
                            start=(m == 0), stop=(m == QLT - 1))
                    nc.vector.tensor_copy(qn_T[h][:, csl], ps[:])
                ps = pp.tile([128, CH], F32, tag="ups")
                for m in range(QLT):
                    nc.tensor.matmul(ps[:],
                                     wqr_s[:, m * HPC * 64:(m + 1) * HPC * 64],
                                     lq[:, m * CH:(m + 1) * CH],
                                     start=(m == 0), stop=(m == QLT - 1))
                nc.vector.tensor_copy(qr2_T[:, csl], ps[:])
                for h in range(HPC):
                    ps = pp.tile([128, CH], F32, tag="ups")
                    for m in range(KVT):
                        nc.tensor.matmul(
                            ps[:],
                            wkn_s[:, m * HPC * NOPE + h * NOPE:
                                  m * HPC * NOPE + (h + 1) * NOPE],
                            lk[:, m * CH:(m + 1) * CH],
                            start=(m == 0), stop=(m == KVT - 1))
                    nc.vector.tensor_copy(kn_T[h][:, csl], ps[:])
                for st in range(CH // 128):
                    ps = pp.tile([128, HPC * VH], F32, tag="ups")
                    for m in range(KVT):
                        nc.tensor.matmul(
                            ps[:],
                            lk[:, m * CH + st * 128:m * CH + (st + 1) * 128],
                            wkv_s[:, m * HPC * VH:(m + 1) * HPC * VH],
                            start=(m == 0), stop=(m == KVT - 1))
                    gst = c * (CH // 128) + st
                    nc.vector.tensor_copy(
                        v2[:, gst * HPC * VH:(gst + 1) * HPC * VH], ps[:])

            # ---- phase 2+3: attention and o-proj, per query chunk ----
            for qc in range(NQC):
                qsl = slice(qc * QC, (qc + 1) * QC)
                mts = []
                for kt in range(ST):
                    mt = mp.tile([128, QC], BF16, tag="mask")
                    nc.sync.dma_start(mt[:],
                                      maskT[kt * 128:(kt + 1) * 128, qsl])
                    mts.append(mt)
                ot = []
                for h in range(HPC):
                    ps_den = pdp.tile([1, QC], F32, tag="den")
                    ps_o = pop.tile([128, QC], F32, tag="po")
                    esum_d = tp.tile([128, QC], F32R, tag="esum_d")
                    for kt in range(ST):
                        ps_s = pp.tile([128, QC], F32, tag="ups")
                        nc.tensor.matmul(ps_s[:],
                                         kn_T[h][:, kt * 128:(kt + 1) * 128],
                                         qn_T[h][:, qsl],
                                         start=True, stop=False)
                        nc.tensor.matmul(
                            ps_s[:],
                            kr2_T[h * 64:(h + 1) * 64,
                                  kt * 128:(kt + 1) * 128],
                            qr2_T[h * 64:(h + 1) * 64, qsl],
                            start=False, stop=True)
                        nc.vector.tensor_add(ps_s[:], ps_s[:], mts[kt][:])
                        et = ep.tile([128, QC], BF16, tag="et")
                        nc.scalar.activation(
                            et[:], ps_s[:], mybir.ActivationFunctionType.Exp,
                            bias=zb[:], scale=1.0)
                        # denominator partials off the PE (DVE accumulate)
                        if kt == 0:
                            nc.vector.tensor_copy(esum_d[:], et[:])
                        else:
                            nc.vector.tensor_add(esum_d[:], esum_d[:], et[:])
                        nc.tensor.matmul(
                            ps_o[:],
                            v2[:, kt * HPC * VH + h * VH:
                               kt * HPC * VH + (h + 1) * VH],
                            et[:], start=(kt == 0), stop=(kt == ST - 1))
                    nc.tensor.matmul(ps_den[:], ones, esum_d[:],
                                     start=True, stop=True)
                    rd = tp.tile([1, QC], F32, tag="rd")
                    dencp = tp.tile([1, QC], F32, tag="dencp")
                    nc.vector.tensor_copy(dencp[:], ps_den[:])
                    nc.vector.reciprocal_approx_fast(rd[:], dencp[:])
                    rdb = tp.tile([128, QC], F32, tag="rdb")
                    nc.gpsimd.partition_broadcast(rdb[:], rd[:1])
                    o = op.tile([128, QC], F32R, tag=f"ot{h}")
                    nc.vector.tensor_mul(o[:], ps_o[:], rdb[:])
                    ot.append(o)
                for st in range(QC // 128):
                    for nn in range(HID // 512):
                        ps_f = pop.tile([128, 512], F32, tag="pf")
                        for h in range(HPC):
                            nc.tensor.matmul(
                                ps_f[:],
                                ot[h][:, st * 128:(st + 1) * 128],
                                wo_s[:, h * HID + nn * 512:
                                     h * HID + (nn + 1) * 512],
                                start=(h == 0), stop=(h == HPC - 1))
                        fo = op.tile([128, 512], F32, tag="fo")
                        nc.scalar.copy(fo[:], ps_f[:])
                        nc.sync.dma_start(
                            part[qc * QC + st * 128:qc * QC + (st + 1) * 128,
                                 nn * 512:(nn + 1) * 512], fo[:])
    nc.compile()
    return nc


def _get(name):
    if name not in _CACHE:
        _CACHE[name] = {"a": _build_a, "b": _build_b}[name]()
    return _CACHE[name]


def _prep(hidden_states, attention_mask, Wqa, gqa, Wqb, Wkva, gkva, Wkvb, Wo):
    f = np.float32
    hid_T = np.ascontiguousarray(hidden_states[0].T, dtype=f)
    import ml_dtypes
    mask_T = np.ascontiguousarray(attention_mask[0, 0].T).astype(ml_dtypes.bfloat16)
    Wqb_g = (np.asarray(gqa, f)[:, None] * np.asarray(Wqb, f)).astype(f)
    Wkvb_g = (np.asarray(gkva, f)[:, None] * np.asarray(Wkvb, f)).astype(f)
    ins_a, ins_b = [], []
    for c in range(NCORES):
        ins_a.append({
            "hsl": np.ascontiguousarray(hid_T[:, c * SL:(c + 1) * SL]),
            "wqa": np.ascontiguousarray(np.asarray(Wqa, f)),
            "wkva": np.ascontiguousarray(np.asarray(Wkva, f)),
        })
        heads = [HPC * c + h for h in range(HPC)]
        wqn = np.concatenate([Wqb_g[:, h * 192:h * 192 + NOPE] for h in heads],
                             axis=1)
        wqr = np.concatenate([Wqb_g[:, h * 192 + NOPE:(h + 1) * 192]
                              for h in heads], axis=1)
        wkn = np.concatenate([Wkvb_g[:, h * 256:h * 256 + NOPE]
                              for h in heads], axis=1)
        wkv = np.concatenate([Wkvb_g[:, h * 256 + NOPE:(h + 1) * 256]
                              for h in heads], axis=1)
        wo = np.concatenate([np.asarray(Wo, f)[h * VH:(h + 1) * VH, :]
                             for h in heads], axis=0)
        ins_b.append({
            "maskT": mask_T,
            "wqn": np.ascontiguousarray(wqn).astype(ml_dtypes.bfloat16),
            "wqr": np.ascontiguousarray(wqr).astype(ml_dtypes.bfloat16),
            "wkn": np.ascontiguousarray(wkn).astype(ml_dtypes.bfloat16),
            "wkv": np.ascontiguousarray(wkv).astype(ml_dtypes.bfloat16),
            "wo": np.ascontiguousarray(wo),
        })
    return ins_a, ins_b


def _run(ins_a, ins_b, trace=False):
    core_ids = list(range(NCORES))
    res_a = run_bass_kernel_spmd(_get("a"), ins_a, core_ids, trace=trace)
    qlat = np.concatenate([res_a.results[c]["q_lat"] for c in range(NCORES)],
                          axis=1)
    kvlat = np.concatenate([res_a.results[c]["kv_lat"] for c in range(NCORES)],
                           axis=1)
    rplat = np.concatenate([res_a.results[c]["rp_lat"] for c in range(NCORES)],
                           axis=1)
    import ml_dtypes
    for m in ins_b:
        m["qlat"] = qlat.astype(ml_dtypes.bfloat16)
        m["kvlat"] = kvlat.astype(ml_dtypes.bfloat16)
        m["rp"] = rplat.astype(ml_dtypes.bfloat16)
    res_b = run_bass_kernel_spmd(_get("b"), ins_b, core_ids, trace=trace)
    out = res_b.results[0]["part"]
    for c in range(1, NCORES):
        out = out + res_b.results[c]["part"]
    return out.astype(np.float32)[None], res_a, res_b


def kernel(hidden_states, attention_mask, Wqa, gqa, Wqb, Wkva, gkva, Wkvb, Wo):
    ins_a, ins_b = _prep(hidden_states, attention_mask, Wqa, gqa, Wqb,
                         Wkva, gkva, Wkvb, Wo)
    out, _, _ = _run(ins_a, ins_b)
    return out



# revision 6
# speedup vs baseline: 1.6742x; 1.6742x over previous
"""DeepSeek-V3.2 MLA attention on 8 Trainium2 NeuronCores (Bass/Tile).

Strategy (tensor parallel over heads per the sharding hint, causal skip):
  Launch A: sequence-sharded latent projections, fp16 in/out, with all
    weights host-packed into the exact SBUF layout (contiguous DMAs).
    Core c computes q/kv down-projections + RMSNorm for its 256-token
    slice in feature-major layout.
  Launch B: head-sharded attention. Core c owns heads (2c, 2c+1).
    The causal structure of the mask is exploited at 128-row block
    granularity: fully-masked key blocks are skipped, the additive mask
    is applied only on the 4 diagonal blocks of each 512-query chunk
    (loaded from a host-packed diagonal band). Per-chunk emission order
    upproj(qc) -> attention(qc) -> o_proj(qc) lets scalar-engine exps
    hide under tensor-engine up-projection work of later chunks.
    Partial outputs are written fp16 and summed on host (the all-reduce
    after o_proj).

Host-side precomputation folds gqa/gkva into Wqb/Wkvb rows, the softmax
1/sqrt(192) into the q-latent normalization, and lays out weights/mask
(layout prep only - all FLOPs of the module run on device).
"""

import numpy as np

import concourse.bass as bass
import concourse.tile as tile
from concourse import bacc, mybir
from concourse.bass_utils import run_bass_kernel_spmd

F32 = mybir.dt.float32
F32R = mybir.dt.float32r
BF16 = mybir.dt.bfloat16
FP16 = mybir.dt.float16

S = 2048
HID = 2048
QL = 1536
KVL = 512
ROPE = 64
NOPE = 128
VH = 128
NH = 16
NCORES = 8
HPC = NH // NCORES          # heads per core = 2
SL = S // NCORES            # token slice per core in launch A = 256
QLT = QL // 128             # 12
KVT = KVL // 128            # 4
HT = HID // 128             # 16
ST = S // 128               # 16
QC = 512                    # attention query chunk
NQC = S // QC               # 4
KPQ = QC // 128             # diagonal k-blocks per query chunk = 4
EPS = 1e-6

_CACHE = {}


def _build_a():
    """Launch A: latents for a 256-token slice, feature-major, fp16.

    in : hsl [128, 16*256] (hidden^T slice, j-tiles side by side)
         wqa [128, 12*16*128] (m-major blocks), wkva [128, 4*16*128],
         wkvr [128, 16*64]
    out: q_lat [QL, SL] = rmsnorm(hidden@Wqa) / sqrt(192)
         kv_lat [KVL, SL], rp_lat [ROPE, SL]
    """
    nc = bacc.Bacc("TRN2", target_bir_lowering=False, debug=False,
                   num_devices=NCORES)
    hsl = nc.dram_tensor("hsl", [128, HT * SL], FP16,
                         kind="ExternalInput").ap()
    wqa = nc.dram_tensor("wqa", [128, QLT * HT * 128], FP16,
                         kind="ExternalInput").ap()
    wkva = nc.dram_tensor("wkva", [128, KVT * HT * 128], FP16,
                          kind="ExternalInput").ap()
    wkvr = nc.dram_tensor("wkvr", [128, HT * ROPE], FP16,
                          kind="ExternalInput").ap()
    q_lat = nc.dram_tensor("q_lat", [QL, SL], FP16,
                           kind="ExternalOutput").ap()
    kv_lat = nc.dram_tensor("kv_lat", [KVL, SL], FP16,
                            kind="ExternalOutput").ap()
    rp_lat = nc.dram_tensor("rp_lat", [ROPE, SL], FP16,
                            kind="ExternalOutput").ap()

    with tile.TileContext(nc) as tc:
        with tc.tile_pool(name="w", bufs=1) as wp, \
             tc.tile_pool(name="h", bufs=1) as hp, \
             tc.tile_pool(name="lat", bufs=1) as lp, \
             tc.tile_pool(name="tmp", bufs=3) as tp, \
             tc.tile_pool(name="out", bufs=1) as op_, \
             tc.tile_pool(name="ps", bufs=2, space="PSUM") as pp, \
             tc.tile_pool(name="pss", bufs=2, space="PSUM") as psp:
            ht = hp.tile([128, HT * SL], FP16, tag="ht")
            nc.sync.dma_start(ht[:], hsl)
            htt = [ht[:, j * SL:(j + 1) * SL] for j in range(HT)]
            # weights: m-block at a time so compute starts early
            wqa_s = wp.tile([128, QLT * HT * 128], FP16, tag="wqa")
            for m in range(QLT):
                nc.sync.dma_start(
                    wqa_s[:, m * HT * 128:(m + 1) * HT * 128],
                    wqa[:, m * HT * 128:(m + 1) * HT * 128])
            wkva_s = wp.tile([128, KVT * HT * 128], FP16, tag="wkva")
            for m in range(KVT):
                nc.sync.dma_start(
                    wkva_s[:, m * HT * 128:(m + 1) * HT * 128],
                    wkva[:, m * HT * 128:(m + 1) * HT * 128])
            wkvr_s = wp.tile([128, HT * ROPE], FP16, tag="wkvr")
            nc.sync.dma_start(wkvr_s[:], wkvr)

            ones_f = wp.tile([128, 1], F32, tag="ones")
            nc.vector.memset(ones_f[:], 1.0)
            ones = ones_f[:].bitcast(F32R)
            epsq = wp.tile([1, 1], F32, tag="epsq")
            nc.vector.memset(epsq[:], 192.0 * EPS)
            epsk = wp.tile([1, 1], F32, tag="epsk")
            nc.vector.memset(epsk[:], EPS)

            qno = op_.tile([128, QLT * SL], FP16, tag="qno")
            kvo = op_.tile([128, KVT * SL], FP16, tag="kvo")

            def down_path(n_tiles, w_s, ssq_scale, eps_ap, out_sb, pfx):
                raw = []
                ps_ssq = psp.tile([1, SL], F32, tag="ssq")
                for m in range(n_tiles):
                    ps = pp.tile([128, SL], F32, tag="dps")
                    for j in range(HT):
                        nc.tensor.matmul(
                            ps[:],
                            w_s[:, (m * HT + j) * 128:(m * HT + j + 1) * 128],
                            htt[j], start=(j == 0), stop=(j == HT - 1))
                    r = lp.tile([128, SL], F32R, tag=f"raw{pfx}{m}")
                    nc.vector.tensor_copy(r[:], ps[:])
                    raw.append(r)
                    sq = tp.tile([128, SL], F32R, tag="sq")
                    nc.scalar.square(sq[:], ps[:])
                    nc.tensor.matmul(ps_ssq[:], ones, sq[:],
                                     start=(m == 0), stop=(m == n_tiles - 1))
                sd = tp.tile([1, SL], F32, tag="sd")
                nc.scalar.activation(sd[:], ps_ssq[:],
                                     mybir.ActivationFunctionType.Sqrt,
                                     bias=eps_ap[:], scale=ssq_scale)
                rr = tp.tile([1, SL], F32, tag="rr")
                nc.vector.reciprocal_approx_fast(rr[:], sd[:])
                rb = tp.tile([128, SL], F32, tag="rb")
                nc.gpsimd.partition_broadcast(rb[:], rr[:1])
                for m in range(n_tiles):
                    nc.vector.tensor_mul(out_sb[:, m * SL:(m + 1) * SL],
                                         raw[m][:], rb[:])

            # q: fold softmax scale 1/sqrt(192) into the rmsnorm scale:
            #   r = 1/sqrt(192*(ssq/QL + eps)) = 1/sqrt(ssq*(192/QL) + 192*eps)
            down_path(QLT, wqa_s, 192.0 / QL, epsq, qno, "q")
            down_path(KVT, wkva_s, 1.0 / KVL, epsk, kvo, "k")
            # raw shared rope part (no norm)
            ps = pp.tile([64, SL], F32, tag="rps")
            for j in range(HT):
                nc.tensor.matmul(
                    ps[:], wkvr_s[:, j * ROPE:(j + 1) * ROPE],
                    htt[j], start=(j == 0), stop=(j == HT - 1))
            ro = tp.tile([64, SL], FP16, tag="ro")
            nc.vector.tensor_copy(ro[:], ps[:])
            nc.sync.dma_start(rp_lat[:, :], ro[:])
            for m in range(QLT):
                nc.sync.dma_start(q_lat[m * 128:(m + 1) * 128, :],
                                  qno[:, m * SL:(m + 1) * SL])
            for m in range(KVT):
                nc.sync.dma_start(kv_lat[m * 128:(m + 1) * 128, :],
                                  kvo[:, m * SL:(m + 1) * SL])
    nc.compile()
    return nc


def _build_b():
    """Launch B: 2 heads of causal attention + o-proj partial, full seq.

    in : qlat [128, 12*2048], kvlat [128, 4*2048], rp [64, 2048] (fp16,
         feature-major latents packed partition-first),
         mdiag [128, 16*512] bf16 (diagonal mask band, [qc*4+t] blocks),
         wqn [128, 12*256], wqr [128, 12*128], wkn [128, 4*256],
         wkv [128, 4*256], wo [128, 2*2048] (fp16, SBUF layouts)
    out: part [S, HID] fp16 (this core's 2-head contribution)
    """
    nc = bacc.Bacc("TRN2", target_bir_lowering=False, debug=False,
                   num_devices=NCORES)
    qlat = nc.dram_tensor("qlat", [128, QLT * S], FP16,
                          kind="ExternalInput").ap()
    kvlat = nc.dram_tensor("kvlat", [128, KVT * S], FP16,
                           kind="ExternalInput").ap()
    rp = nc.dram_tensor("rp", [ROPE, S], FP16, kind="ExternalInput").ap()
    mdiag = nc.dram_tensor("mdiag", [128, ST * QC], BF16,
                           kind="ExternalInput").ap()
    wqn = nc.dram_tensor("wqn", [128, QLT * HPC * NOPE], FP16,
                         kind="ExternalInput").ap()
    wqr = nc.dram_tensor("wqr", [128, QLT * HPC * 64], FP16,
                         kind="ExternalInput").ap()
    wkn = nc.dram_tensor("wkn", [128, KVT * HPC * NOPE], FP16,
                         kind="ExternalInput").ap()
    wkv = nc.dram_tensor("wkv", [128, KVT * HPC * VH], FP16,
                         kind="ExternalInput").ap()
    wo = nc.dram_tensor("wo", [128, HPC * HID], FP16,
                        kind="ExternalInput").ap()
    part = nc.dram_tensor("part", [S, HID], FP16, kind="ExternalOutput").ap()

    CH = QC             # up-projection chunk == query chunk (512)

    with tile.TileContext(nc) as tc:
        with tc.tile_pool(name="w", bufs=1) as wp, \
             tc.tile_pool(name="act", bufs=1) as ap_, \
             tc.tile_pool(name="lq", bufs=2) as lqp, \
             tc.tile_pool(name="tmp", bufs=2) as tp, \
             tc.tile_pool(name="et", bufs=6) as ep, \
             tc.tile_pool(name="es", bufs=2) as esp, \
             tc.tile_pool(name="out", bufs=3) as op, \
             tc.tile_pool(name="ps", bufs=2, space="PSUM") as pp, \
             tc.tile_pool(name="psc", bufs=2, space="PSUM") as pcp, \
             tc.tile_pool(name="psden", bufs=1, space="PSUM") as pdp, \
             tc.tile_pool(name="pso", bufs=1, space="PSUM") as pop, \
             tc.tile_pool(name="psf", bufs=2, space="PSUM") as pfp:
            ones_h = wp.tile([128, 1], FP16, tag="ones")
            nc.vector.memset(ones_h[:], 1.0)

            # ---- persistent per-head activations (feature-major fp16) ----
            qn_T = [ap_.tile([128, S], FP16, tag=f"qnT{h}", name=f"qnT{h}")
                    for h in range(HPC)]
            qr2_T = ap_.tile([128, S], FP16, tag="qr2T")
            kn_T = [ap_.tile([128, S], FP16, tag=f"knT{h}", name=f"knT{h}")
                    for h in range(HPC)]
            v2 = ap_.tile([128, ST * HPC * VH], FP16, tag="v2")
            kr2_T = ap_.tile([128, S], FP16, tag="kr2T")

            def load_chunk(c):
                csl = slice(c * CH, (c + 1) * CH)
                lq = lqp.tile([128, QLT * CH], FP16, tag="lq", name="lq")
                nc.sync.dma_start(
                    lq[:].rearrange("p (m s) -> p m s", m=QLT),
                    qlat.rearrange("p (m s) -> p m s", m=QLT)[:, :, csl])
                lk = lqp.tile([128, KVT * CH], FP16, tag="lk", name="lk")
                nc.sync.dma_start(
                    lk[:].rearrange("p (m s) -> p m s", m=KVT),
                    kvlat.rearrange("p (m s) -> p m s", m=KVT)[:, :, csl])
                nc.sync.dma_start(kr2_T[0:64, csl], rp[:, csl])
                nc.sync.dma_start(kr2_T[64:128, csl], rp[:, csl])
                return lq, lk

            pend = load_chunk(0)
            # ---- weights + mask band to SBUF (after first latent chunk) ----
            wqn_s = wp.tile([128, QLT * HPC * NOPE], FP16, tag="wqn")
            nc.sync.dma_start(wqn_s[:], wqn)
            wqr_s = wp.tile([128, QLT * HPC * 64], FP16, tag="wqr")
            nc.sync.dma_start(wqr_s[:], wqr)
            wkn_s = wp.tile([128, KVT * HPC * NOPE], FP16, tag="wkn")
            nc.sync.dma_start(wkn_s[:], wkn)
            wkv_s = wp.tile([128, KVT * HPC * VH], FP16, tag="wkv")
            nc.sync.dma_start(wkv_s[:], wkv)
            wo_s = wp.tile([128, HPC * HID], FP16, tag="wo")
            nc.sync.dma_start(wo_s[:], wo)
            mdg = wp.tile([128, ST * QC], BF16, tag="mdg")
            nc.sync.dma_start(mdg[:], mdiag)

            for qc in range(NQC):
                csl = slice(qc * CH, (qc + 1) * CH)
                lq, lk = pend
                if qc + 1 < NQC:
                    pend = load_chunk(qc + 1)

                # ---- phase 1 for this chunk: up-projections ----
                for h in range(HPC):
                    ps = pp.tile([128, CH], F32, tag="ups")
                    for m in range(QLT):
                        nc.tensor.matmul(
                            ps[:],
                            wqn_s[:, m * HPC * NOPE + h * NOPE:
                                  m * HPC * NOPE + (h + 1) * NOPE],
                            lq[:, m * CH:(m + 1) * CH],
                            start=(m == 0), stop=(m == QLT - 1))
                    nc.vector.tensor_copy(qn_T[h][:, csl], ps[:])
                ps = pp.tile([128, CH], F32, tag="ups")
                for m in range(QLT):
                    nc.tensor.matmul(ps[:],
                                     wqr_s[:, m * HPC * 64:(m + 1) * HPC * 64],
                                     lq[:, m * CH:(m + 1) * CH],
                                     start=(m == 0), stop=(m == QLT - 1))
                nc.vector.tensor_copy(qr2_T[:, csl], ps[:])
                for h in range(HPC):
                    ps = pp.tile([128, CH], F32, tag="ups")
                    for m in range(KVT):
                        nc.tensor.matmul(
                            ps[:],
                            wkn_s[:, m * HPC * NOPE + h * NOPE:
                                  m * HPC * NOPE + (h + 1) * NOPE],
                            lk[:, m * CH:(m + 1) * CH],
                            start=(m == 0), stop=(m == KVT - 1))
                    nc.vector.tensor_copy(kn_T[h][:, csl], ps[:])
                for st in range(CH // 128):
                    ps = pp.tile([128, HPC * VH], F32, tag="ups")
                    for m in range(KVT):
                        nc.tensor.matmul(
                            ps[:],
                            lk[:, m * CH + st * 128:m * CH + (st + 1) * 128],
                            wkv_s[:, m * HPC * VH:(m + 1) * HPC * VH],
                            start=(m == 0), stop=(m == KVT - 1))
                    gst = qc * (CH // 128) + st
                    nc.vector.tensor_copy(
                        v2[:, gst * HPC * VH:(gst + 1) * HPC * VH], ps[:])

                # ---- phase 2: causal attention for this query chunk ----
                nkt = KPQ * (qc + 1)       # causal: k blocks 0..nkt-1
                ot = []
                for h in range(HPC):
                    ps_o = pop.tile([128, QC], F32, tag="po")
                    esum = esp.tile([128, QC], FP16, tag="esum")
                    prev_et = None
                    for kt in range(nkt):
                        ps_s = pcp.tile([128, QC], F32, tag="scs")
                        nc.tensor.matmul(
                            ps_s[:],
                            kn_T[h][:, kt * 128:(kt + 1) * 128],
                            qn_T[h][:, csl], start=True, stop=False)
                        nc.tensor.matmul(
                            ps_s[:],
                            kr2_T[h * 64:(h + 1) * 64,
                                  kt * 128:(kt + 1) * 128],
                            qr2_T[h * 64:(h + 1) * 64, csl],
                            start=False, stop=True)
                        d = kt - KPQ * qc
                        if d >= 0:   # diagonal block: apply mask values
                            nc.vector.tensor_add(
                                ps_s[:], ps_s[:],
                                mdg[:, (qc * KPQ + d) * QC:
                                    (qc * KPQ + d + 1) * QC])
                        et = ep.tile([128, QC], FP16, tag="et")
                        nc.scalar.activation(
                            et[:], ps_s[:],
                            mybir.ActivationFunctionType.Exp)
                        if kt == 0:
                            nc.vector.tensor_copy(esum[:], et[:])
                        else:
                            nc.vector.tensor_add(esum[:], esum[:], et[:])
                        # PV software-pipelined one block behind the exps
                        if prev_et is not None:
                            nc.tensor.matmul(
                                ps_o[:],
                                v2[:, (kt - 1) * HPC * VH + h * VH:
                                   (kt - 1) * HPC * VH + (h + 1) * VH],
                                prev_et[:], start=(kt == 1), stop=False)
                        prev_et = et
                    nc.tensor.matmul(
                        ps_o[:],
                        v2[:, (nkt - 1) * HPC * VH + h * VH:
                           (nkt - 1) * HPC * VH + (h + 1) * VH],
                        prev_et[:], start=(nkt == 1), stop=True)
                    ps_den = pdp.tile([1, QC], F32, tag="den")
                    nc.tensor.matmul(ps_den[:], ones_h[:], esum[:],
                                     start=True, stop=True)
                    dencp = tp.tile([1, QC], F32, tag="dencp")
                    nc.vector.tensor_copy(dencp[:], ps_den[:])
                    rd = tp.tile([1, QC], F32, tag="rd")
                    nc.vector.reciprocal_approx_fast(rd[:], dencp[:])
                    rdb = tp.tile([128, QC], F32, tag="rdb")
                    nc.gpsimd.partition_broadcast(rdb[:], rd[:1])
                    o = op.tile([128, QC], FP16, tag=f"ot{h}")
                    nc.vector.tensor_mul(o[:], ps_o[:], rdb[:])
                    ot.append(o)

                # ---- phase 3: o-proj partial for this query chunk ----
                for st in range(QC // 128):
                    fo = op.tile([128, HID], FP16, tag="fo")
                    for nn in range(HID // 512):
                        ps_f = pfp.tile([128, 512], F32, tag="pf")
                        for h in range(HPC):
                            nc.tensor.matmul(
                                ps_f[:],
                                ot[h][:, st * 128:(st + 1) * 128],
                                wo_s[:, h * HID + nn * 512:
                                     h * HID + (nn + 1) * 512],
                                start=(h == 0), stop=(h == HPC - 1))
                        nc.vector.tensor_copy(
                            fo[:, nn * 512:(nn + 1) * 512], ps_f[:])
                    nc.sync.dma_start(
                        part[qc * QC + st * 128:qc * QC + (st + 1) * 128, :],
                        fo[:])
    nc.compile()
    return nc


def _get(name):
    if name not in _CACHE:
        _CACHE[name] = {"a": _build_a, "b": _build_b}[name]()
    return _CACHE[name]


def _prep(hidden_states, attention_mask, Wqa, gqa, Wqb, Wkva, gkva, Wkvb, Wo):
    f16 = np.float16
    f = np.float32
    import ml_dtypes
    hid_T = np.ascontiguousarray(hidden_states[0].T, dtype=f)  # [HID, S]
    Wqa_f = np.asarray(Wqa, f)
    Wkva_f = np.asarray(Wkva, f)
    Wqb_g = (np.asarray(gqa, f)[:, None] * np.asarray(Wqb, f))
    Wkvb_g = (np.asarray(gkva, f)[:, None] * np.asarray(Wkvb, f))
    Wo_f = np.asarray(Wo, f)
    mask = np.asarray(attention_mask[0, 0], f)                 # [q, k]

    # launch-A weights (same for all cores), packed to SBUF layout
    wqa_p = np.ascontiguousarray(
        Wqa_f.reshape(HT, 128, QLT, 128).transpose(1, 2, 0, 3)
        .reshape(128, QLT * HT * 128)).astype(f16)
    wkva_p = np.ascontiguousarray(
        Wkva_f[:, :KVL].reshape(HT, 128, KVT, 128).transpose(1, 2, 0, 3)
        .reshape(128, KVT * HT * 128)).astype(f16)
    wkvr_p = np.ascontiguousarray(
        Wkva_f[:, KVL:].reshape(HT, 128, ROPE).transpose(1, 0, 2)
        .reshape(128, HT * ROPE)).astype(f16)

    # diagonal mask band (same for all cores)
    mdiag = np.zeros((128, ST * QC), dtype=f)
    for qcb in range(NQC):
        blk = mask[qcb * QC:(qcb + 1) * QC, qcb * QC:(qcb + 1) * QC].T
        for t in range(KPQ):
            mdiag[:, (qcb * KPQ + t) * QC:(qcb * KPQ + t + 1) * QC] = \
                blk[t * 128:(t + 1) * 128, :]
    mdiag = mdiag.astype(ml_dtypes.bfloat16)

    def pack_cols(w, n_tiles):
        """[n_tiles*128, C] -> [128, n_tiles*C] partition-first."""
        c = w.shape[1]
        return np.ascontiguousarray(
            w.reshape(n_tiles, 128, c).transpose(1, 0, 2)
            .reshape(128, n_tiles * c)).astype(f16)

    ins_a, ins_b = [], []
    for c in range(NCORES):
        hsl = np.ascontiguousarray(
            hid_T[:, c * SL:(c + 1) * SL].reshape(HT, 128, SL)
            .transpose(1, 0, 2).reshape(128, HT * SL)).astype(f16)
        ins_a.append({"hsl": hsl, "wqa": wqa_p, "wkva": wkva_p,
                      "wkvr": wkvr_p})
        heads = [HPC * c + h for h in range(HPC)]
        wqn = pack_cols(np.concatenate(
            [Wqb_g[:, h * 192:h * 192 + NOPE] for h in heads], axis=1), QLT)
        wqr = pack_cols(np.concatenate(
            [Wqb_g[:, h * 192 + NOPE:(h + 1) * 192] for h in heads],
            axis=1), QLT)
        wkn = pack_cols(np.concatenate(
            [Wkvb_g[:, h * 256:h * 256 + NOPE] for h in heads], axis=1), KVT)
        wkv = pack_cols(np.concatenate(
            [Wkvb_g[:, h * 256 + NOPE:(h + 1) * 256] for h in heads],
            axis=1), KVT)
        wo = np.concatenate(
            [Wo_f[h * VH:(h + 1) * VH, :] for h in heads],
            axis=1)  # [128, 2*HID]
        ins_b.append({
            "mdiag": mdiag, "wqn": wqn, "wqr": wqr, "wkn": wkn, "wkv": wkv,
            "wo": np.ascontiguousarray(wo).astype(f16),
        })
    return ins_a, ins_b


def _run(ins_a, ins_b, trace=False):
    core_ids = list(range(NCORES))
    res_a = run_bass_kernel_spmd(_get("a"), ins_a, core_ids, trace=trace)
    qlat = np.concatenate([res_a.results[c]["q_lat"] for c in range(NCORES)],
                          axis=1)
    kvlat = np.concatenate([res_a.results[c]["kv_lat"]
                            for c in range(NCORES)], axis=1)
    rplat = np.concatenate([res_a.results[c]["rp_lat"]
                            for c in range(NCORES)], axis=1)
    qlat_p = np.ascontiguousarray(
        qlat.reshape(QLT, 128, S).transpose(1, 0, 2).reshape(128, QLT * S))
    kvlat_p = np.ascontiguousarray(
        kvlat.reshape(KVT, 128, S).transpose(1, 0, 2).reshape(128, KVT * S))
    for m in ins_b:
        m["qlat"] = qlat_p
        m["kvlat"] = kvlat_p
        m["rp"] = np.ascontiguousarray(rplat)
    res_b = run_bass_kernel_spmd(_get("b"), ins_b, core_ids, trace=trace)
    out = res_b.results[0]["part"].astype(np.float32)
    for c in range(1, NCORES):
        out = out + res_b.results[c]["part"].astype(np.float32)
    return out[None], res_a, res_b


def kernel(hidden_states, attention_mask, Wqa, gqa, Wqb, Wkva, gkva, Wkvb, Wo):
    ins_a, ins_b = _prep(hidden_states, attention_mask, Wqa, gqa, Wqb,
                         Wkva, gkva, Wkvb, Wo)
    out, _, _ = _run(ins_a, ins_b)
    return out
